# revision 68
# baseline (speedup 1.0000x reference)
"""Trainium2 Bass kernel for nn_Autoencoder (point-cloud GNN autoencoder).

Data-parallel over batch: 8 point clouds -> 8 NeuronCores. Each core runs the
full pipeline for one cloud: kNN (distance matmul + iterative top-k on the
vector engine), then 4 graph-conv layers with AdaIN, using per-rank indirect
DMA gathers for neighbor features.
"""
import sys, types, ctypes, contextlib
sys.path.insert(0, '/opt/trn_rl_repo')

import numpy as np
import bass_rust
from concourse import bass, mybir
from concourse.tile import TileContext
from concourse.masks import make_identity
from concourse.library_config import mlp
from concourse.library_overlay import lower_extended_insts

B, V, NB, SUP = 8, 2048, 20, 4
NT = V // 128  # 16 point tiles per core
F32 = mybir.dt.float32
BF16 = mybir.dt.bfloat16
FP16 = mybir.dt.float16
I32 = mybir.dt.int32
I16 = mybir.dt.int16
U32 = mybir.dt.uint32
U16 = mybir.dt.uint16
AF = mybir.ActivationFunctionType
ALU = mybir.AluOpType


def _split_excess_waits(nc, max_waits=1):
    """Walrus here rejects >1 sync waits per instruction; move extras onto
    NOPs on the same engine right before it."""
    for f in nc.m.functions:
        for bb in f.blocks:
            insts = list(bb.instructions)
            out = []
            for inst in insts:
                si = getattr(inst, 'sync_info', None)
                if si is not None and si.on_wait and len(si.on_wait) > max_waits:
                    waits = list(si.on_wait)
                    move, keep = waits[:-max_waits], waits[-max_waits:]
                    for w in move:
                        eng = nc.engines[inst.engine]
                        nop = eng.nop(nofuse=True)
                        ni = nop.ins
                        for f2 in nc.m.functions:
                            for bb2 in f2.blocks:
                                if ni in bb2.instructions:
                                    bb2.instructions.remove(ni)
                        ni.sync_info = bass_rust.SyncInfo(on_wait=[w], on_update=[])
                        out.append(ni)
                    si.on_wait = keep
                out.append(inst)
            bb.instructions[:] = out


def _normalize_cols(d):
    n = np.sqrt((d.astype(np.float32) ** 2).sum(0))
    return (d / np.maximum(n, 1e-12)).astype(np.float32)


def _block_dirs(dirsn, K):
    """(3, K) normalized dirs -> block-diagonal (60, NB*K) bf16: row (r,d), col (r,k)."""
    import ml_dtypes
    bd = np.zeros((3 * NB, NB * K), np.float32)
    for r in range(NB):
        bd[3 * r:3 * r + 3, K * r:K * (r + 1)] = dirsn
    return bd.astype(ml_dtypes.bfloat16)


def build_kernel():
    nc = bass.Bass()
    src = nc.dram_tensor("source", [V, 3], F32, kind="ExternalInput")
    tf = nc.dram_tensor("target_feature", [V, 10], F32, kind="ExternalInput")
    # host-packed weight constants (block dirs pre-converted to bf16)
    bd0 = nc.dram_tensor("bd0", [60, NB * 64], BF16, kind="ExternalInput")
    bd1 = nc.dram_tensor("bd1", [60, NB * 128], BF16, kind="ExternalInput")
    bd2 = nc.dram_tensor("bd2", [60, NB * 64], BF16, kind="ExternalInput")
    bd3 = nc.dram_tensor("bd3", [60, NB * 12], BF16, kind="ExternalInput")
    wb1 = nc.dram_tensor("wb1", [17, 160], F32, kind="ExternalInput")   # [conv1_w; conv1_b]
    wba = nc.dram_tensor("wba", [11, 64], F32, kind="ExternalInput")    # [adain_w; adain_b]
    wbd1 = nc.dram_tensor("wbd1", [33, 80], F32, kind="ExternalInput")  # [dc1_w; dc1_b]
    wbd2 = nc.dram_tensor("wbd2", [17, 15], F32, kind="ExternalInput")  # [dc2_w; dc2_b]
    out = nc.dram_tensor("out", [V, 3], F32, kind="ExternalOutput")
    # internal DRAM feature tables for gathers (rows = points; dma_gather
    # needs 256B-multiple rows, so narrow tables are padded to 64 f32)
    t_vert = nc.dram_tensor("t_vert", [V, 64], F32)
    t_f1w = nc.dram_tensor("t_f1w", [V, 128], BF16)
    t_tw = nc.dram_tensor("t_tw", [V, 128], BF16)
    t_c1w = nc.dram_tensor("t_c1w", [V, 128], BF16)

    with TileContext(nc) as tc:
        with (
            tc.tile_pool(name="big", bufs=2) as big,       # (128,2048) tiles
            tc.tile_pool(name="mid", bufs=2) as mid,
            tc.tile_pool(name="sgp", bufs=3) as sgp,       # (128,~2560) tiles
            tc.tile_pool(name="sml", bufs=4) as sml,
            tc.tile_pool(name="keep", bufs=1) as keep,     # persistent caches
            tc.tile_pool(name="ps", bufs=2, space="PSUM") as ps,
            tc.tile_pool(name="ps_nd", bufs=1, space="PSUM") as ps_nd,
            tc.tile_pool(name="ps2", bufs=1, space="PSUM") as ps2,
            tc.tile_pool(name="psf", bufs=2, space="PSUM") as psf,
            tc.tile_pool(name="pst", bufs=2, space="PSUM") as pst,
        ):
            nc.gpsimd.load_library(mlp)
            ident = keep.tile([128, 128], F32)
            make_identity(nc, ident[:])

            # verts table padded to 64 f32/row for dma_gather (256B rows)
            nc.sync.dma_start(
                out=bass.AP(t_vert[:].tensor, 0, [[64, V], [1, 3]]), in_=src[:])

            # ---- vertsT (3, 2048) and lhsT/rhs for the distance matmul ----
            vT = keep.tile([3, V], F32)
            nc.sync.dma_start(out=vT[:], in_=bass.AP(src[:].tensor, 0, [[1, 3], [3, V]]))
            vT2 = keep.tile([3, V], F32)
            nc.vector.tensor_mul(out=vT2[:], in0=vT[:], in1=vT[:])
            ones3 = keep.tile([3, 1], F32)
            nc.vector.memset(ones3[:], 1.0)
            sqrow = keep.tile([1, V], F32)
            for j in range(4):
                sq_ps = ps.tile([1, 512], F32, tag="theta")
                nc.tensor.matmul(out=sq_ps[:], lhsT=ones3[:],
                                 rhs=vT2[:, bass.ts(j, 512)], start=True, stop=True)
                nc.scalar.copy(out=sqrow[:, bass.ts(j, 512)], in_=sq_ps[:])
            # lhsT (5, 2048) = [x;y;z; ones; sq] ; rhs (5, 2048) = [2x;2y;2z; -sq; -ones]
            lhsT = keep.tile([5, V], F32)
            rhsd = keep.tile([5, V], F32)
            onesrow = keep.tile([1, V], F32)
            nc.vector.memset(onesrow[:], 1.0)
            negones = keep.tile([1, V], F32)
            nc.vector.memset(negones[:], -1.0)
            negsq = keep.tile([1, V], F32)
            nc.vector.tensor_copy(out=lhsT[:3, :], in_=vT[:])
            nc.sync.dma_start(out=lhsT[3:4, :], in_=onesrow[:])
            nc.sync.dma_start(out=lhsT[4:5, :], in_=sqrow[:])
            nc.vector.tensor_scalar_mul(rhsd[:3, :], vT[:], 2.0)
            nc.sync.dma_start(out=rhsd[4:5, :], in_=negones[:])
            # chunked so the tile-0 dist matmul can start per 512-col chunk
            for j in range(4):
                nc.vector.tensor_scalar_mul(negsq[:, bass.ts(j, 512)],
                                            sqrow[:, bass.ts(j, 512)], -1.0)
                nc.sync.dma_start(out=rhsd[3:4, bass.ts(j, 512)],
                                  in_=negsq[:, bass.ts(j, 512)])

            # persistent caches across passes
            idx16 = keep.tile([128, NT * 20], I16)         # neighbor idx ranks 1-20
            # wrapped idx list for dma_gather: descriptor i of tile t lives at
            # partition i%16 (replicated in all 8 groups), col t*160 + i//16,
            # with i = r*128 + p
            wrap = keep.tile([128, NT * 160], I16)
            dnT_all = keep.tile([60, NT * 128], BF16)      # transposed unit directions
            # feature caches carry a trailing ones column per tile (cin+1 wide)
            # so feat_matmul's transpose picks up the bias row for free
            f1_all = keep.tile([128, NT * 17], F32)
            nc.vector.memset(f1_all[:], 1.0)
            f2_all = keep.tile([128, NT * 32], F32)
            t_all = keep.tile([128, NT * 33], F32)
            nc.vector.memset(t_all[:], 1.0)
            c1_all = keep.tile([128, NT * 17], F32)
            nc.vector.memset(c1_all[:], 1.0)
            hp_all = keep.tile([128, NT * 64], F32)        # adain style proj cache
            vts = keep.tile([128, NT * 3], F32)            # verts per tile (i-major)
            s1acc = keep.tile([1, 32], F32)
            s2acc = keep.tile([1, 32], F32)
            nc.vector.memset(s1acc[:], 0.0)
            nc.vector.memset(s2acc[:], 0.0)

            wb1s = keep.tile([17, 160], F32)
            nc.sync.dma_start(out=wb1s[:], in_=wb1[:])
            wbas = keep.tile([11, 64], F32)
            nc.sync.dma_start(out=wbas[:], in_=wba[:])
            wbd1s = keep.tile([33, 80], F32)
            nc.sync.dma_start(out=wbd1s[:], in_=wbd1[:])
            wbd2s = keep.tile([17, 15], F32)
            nc.sync.dma_start(out=wbd2s[:], in_=wbd2[:])
            bd0s = keep.tile([60, NB * 64], BF16)
            nc.sync.dma_start(out=bd0s[:], in_=bd0[:])
            bd1s = keep.tile([60, NB * 128], BF16)
            nc.sync.dma_start(out=bd1s[:], in_=bd1[:])
            bd2s = keep.tile([60, NB * 64], BF16)
            nc.sync.dma_start(out=bd2s[:], in_=bd2[:])
            bd3s = keep.tile([60, NB * 12], BF16)
            nc.sync.dma_start(out=bd3s[:], in_=bd3[:])

            r1024 = nc.gpsimd.to_reg(1024)
            r512 = nc.gpsimd.to_reg(512)

            def wrap_idx(t):
                """Shuffle tile t's idx16 (128, 20) into the dma_gather wrap
                layout: descriptor i=r*128+p at partition i%16, col i//16,
                replicated across the 8 Q7 core groups. The (r,g)->col
                transpose forces single-element descriptors, one DMA per g."""
                w = wrap[:]
                ix = idx16[:]
                ps_w = w.ap[0][0]
                ps_i = ix.ap[0][0]
                for g in range(8):
                    src = bass.AP(ix.tensor, ix.offset + g * 16 * ps_i + t * 20,
                                  [[ps_i, 16], [1, 20], [1, 1]])
                    dst = bass.AP(w.tensor, w.offset + t * 160 + g,
                                  [[ps_w, 16], [8, 20], [1, 1]])
                    nc.sync.dma_start(out=dst, in_=src)
                # replicate group 0 into groups 1..7 by doubling
                for np_ in (16, 32, 64):
                    dstr = bass.AP(w.tensor, w.offset + np_ * ps_w + t * 160,
                                   [[ps_w, np_], [1, 160]])
                    srcr = bass.AP(w.tensor, w.offset + t * 160,
                                   [[ps_w, np_], [1, 160]])
                    nc.sync.dma_start(out=dstr, in_=srcr)

            def gather_ranks(t, table, C, dest):
                """All 20 neighbor rows per point of tile t via 3 dma_gathers
                (<=1024 idxs each: this ucode crashes above that).
                dest (128, NB*C): dest[p, r*C:(r+1)*C] = table[idx[p,r]]."""
                d3 = dest[:].rearrange("p (r c) -> p r c", r=NB, c=C)
                wt = wrap[:, t * 160:(t + 1) * 160]
                for r0, nr, reg in ((0, 8, r1024), (8, 8, r1024), (16, 4, r512)):
                    nc.gpsimd.dma_gather(
                        d3[:, r0:r0 + nr, :], table[:],
                        wt[:, r0 * 8:(r0 + nr) * 8],
                        nr * 128, reg, C)

            def theta_relu(t, bds, K, dest):
                """dest (128, NB*K) sbuf = relu(dnT_t.T @ block dirs)."""
                n = NB * K
                dT = dnT_all[:, t * 128:(t + 1) * 128]
                for j in range(0, n, 512):
                    w = min(512, n - j)
                    tp = ps.tile([128, 512], F32, tag="theta")
                    nc.tensor.matmul(out=tp[:, :w], lhsT=dT,
                                     rhs=bds[:, j:j + w], start=True, stop=True)
                    nc.scalar.activation(out=dest[:, j:j + w], in_=tp[:, :w], func=AF.Relu)

            def feat_matmul(t, fmap1_ap, cin, wbs, nout, lt_dve=False):
                """feat (128, nout) = [fmap | 1] @ [w; b] for tile t.
                fmap1_ap is (128, cin+1) with a trailing ones column.
                lt_dve: stage the lhsT copy on DVE (for phases where DVE idles
                while Act is on the feat-chain critical path)."""
                ftp = pst.tile([cin + 1, 128], F32, tag="ftp")
                nc.tensor.transpose(out=ftp[:], in_=fmap1_ap, identity=ident[:])
                lt = sml.tile([cin + 1, 128], F32, tag="lt")
                if lt_dve:
                    nc.vector.tensor_copy(out=lt[:], in_=ftp[:])
                else:
                    nc.scalar.copy(out=lt[:], in_=ftp[:])
                fp = psf.tile([128, nout], F32, tag="feat")
                nc.tensor.matmul(out=fp[:], lhsT=lt[:], rhs=wbs[:], start=True, stop=True)
                return fp

            # ================= pass 0: dist + topk + dn + conv0 =================
            for t in range(NT):
                row = big.tile([128, V], F32, tag="row")
                for j in range(4):
                    nd_ps = ps_nd.tile([128, 512], F32, tag="nd")
                    nc.tensor.matmul(out=nd_ps[:],
                                     lhsT=lhsT[:, bass.ts(t, 128)],
                                     rhs=rhsd[:, bass.ts(j, 512)], start=True, stop=True)
                    nc.scalar.copy(out=row[:, bass.ts(j, 512)], in_=nd_ps[:])
                # blocked top-k: per-256-col block top-8 (one full scan), then
                # top-24 among the 64 block candidates, then 3 max_index scans
                # to recover original columns. Misses a far neighbor only when
                # >8 of the true top-21 share one block (~0.4% of points).
                bmax = sml.tile([128, 64], F32, tag="bmax")
                for b in range(8):
                    nc.vector.max(out=bmax[:, b * 8:(b + 1) * 8],
                                  in_=row[:, b * 256:(b + 1) * 256])
                scr64 = sml.tile([128, 64], F32, tag="scr64")
                v8 = sml.tile([128, 24], F32, tag="v8")
                iu = sml.tile([128, 24], U32, tag="iu")
                nc.vector.max(out=v8[:, 0:8], in_=bmax[:])
                nc.vector.match_replace(out=scr64[:], in_to_replace=v8[:, 0:8],
                                        in_values=bmax[:], imm_value=-3.0e38)
                nc.vector.max(out=v8[:, 8:16], in_=scr64[:])
                nc.vector.match_replace(out=scr64[:], in_to_replace=v8[:, 8:16],
                                        in_values=scr64[:], imm_value=-3.0e38)
                nc.vector.max(out=v8[:, 16:24], in_=scr64[:])
                nc.vector.max_index(out=iu[:, 0:8], in_max=v8[:, 0:8], in_values=row[:])
                nc.vector.max_index(out=iu[:, 8:16], in_max=v8[:, 8:16], in_values=row[:])
                nc.vector.max_index(out=iu[:, 16:24], in_max=v8[:, 16:24], in_values=row[:])
                # pack ranks 1..20 (drop rank 0 = self) for batched gathers
                nc.vector.tensor_copy(out=idx16[:, t * 20:t * 20 + 20], in_=iu[:, 1:21])
                wrap_idx(t)

            # ===== pass 0b: gathers + dn + conv0 (split from 0a so the next
            # tile's dist-row Act copies never queue behind gather-latency-
            # bound theta work: that head-of-line block cost ~5us/tile) =====
            for t in range(NT):
                # verts of this tile + rank-gathered neighbor verts
                vt = vts[:, t * 3:(t + 1) * 3]
                nc.sync.dma_start(out=vt, in_=src[t * 128:(t + 1) * 128, :])
                vg64 = sgp.tile([128, NB * 64], F32, tag="sg")
                gather_ranks(t, t_vert, 64, vg64)
                vg = bass.AP(vg64.tensor, vg64[:].offset,
                             [[vg64[:].ap[0][0], 128], [64, NB], [1, 3]])
                # dvec, norms, dn
                dv = mid.tile([128, NB * 3], F32, tag="dv")
                vt_b = bass.AP(vts[:].tensor, vts[:].offset + t * 3,
                               [[NT * 3, 128], [0, NB], [1, 3]])
                dv3 = dv[:].rearrange("p (r d) -> p r d", r=NB, d=3)
                nc.vector.tensor_tensor(out=dv3, in0=vg, in1=vt_b, op=ALU.subtract)
                # nsq must be the exact sum of squares: the matmul-form dist
                # loses ~1e-6 absolute to cancellation, comparable to the
                # smallest neighbor distances in this point cloud
                dsq = mid.tile([128, NB * 3], F32, tag="dsq")
                nc.vector.tensor_mul(out=dsq[:], in0=dv[:], in1=dv[:])
                nsq = sml.tile([128, NB], F32, tag="nsq")
                nc.vector.tensor_reduce(
                    out=nsq[:], in_=dsq[:].rearrange("p (r d) -> p r d", r=NB, d=3),
                    axis=mybir.AxisListType.X, op=ALU.add)
                rn = sml.tile([128, NB], F32, tag="rn")
                nc.scalar.activation(out=rn[:], in_=nsq[:], func=AF.Sqrt)
                nc.vector.tensor_scalar_max(rn[:], rn[:], 1e-12)
                nc.vector.reciprocal(out=rn[:], in_=rn[:])
                dn = mid.tile([128, NB * 3], F32, tag="dn")
                rn_b = bass.AP(rn.tensor, rn[:].offset, [[rn[:].ap[0][0], 128], [1, NB], [0, 3]])
                nc.vector.tensor_tensor(out=dn[:], in0=dv[:], in1=rn_b, op=ALU.mult)
                dnp = pst.tile([60, 128], F32, tag="ftp")
                nc.tensor.transpose(out=dnp[:], in_=dn[:, :60], identity=ident[:])
                nc.scalar.copy(out=dnT_all[:, t * 128:(t + 1) * 128], in_=dnp[:])

                # conv0: theta only, K=64 -> f1
                th0 = mid.tile([128, NB * 64], BF16, tag="th")
                theta_relu(t, bd0s, 64, th0)
                mx = sml.tile([128, 64], BF16, tag="mx64")
                nc.vector.tensor_reduce(
                    out=mx[:], in_=bass.AP(th0.tensor, th0[:].offset,
                                           [[th0[:].ap[0][0], 128], [1, 64], [64, NB]]),
                    axis=mybir.AxisListType.X, op=ALU.max)
                f1t = f1_all[:, t * 17:t * 17 + 16]
                nc.vector.tensor_reduce(
                    out=f1t, in_=bass.AP(mx.tensor, mx[:].offset,
                                         [[mx[:].ap[0][0], 128], [1, 16], [16, 4]]),
                    axis=mybir.AxisListType.X, op=ALU.add)
                nc.vector.tensor_scalar_max(f1t, f1t, 0.0)
                # conv1 feature table rows
                fp = feat_matmul(t, f1_all[:, t * 17:(t + 1) * 17], 16, wb1s[:], 160)
                sup = sml.tile([128, 128], BF16, tag="sup1")
                nc.vector.tensor_copy(out=sup[:], in_=fp[:, 32:160])
                nc.sync.dma_start(out=t_f1w[t * 128:(t + 1) * 128, :], in_=sup[:])

            # ================= pass 1: conv1 -> f2, adain stats =================
            ones128 = keep.tile([128, 1], F32)
            nc.vector.memset(ones128[:], 1.0)
            for t in range(NT):
                sg = sgp.tile([128, NB * 128], BF16, tag="sg")
                gather_ranks(t, t_f1w, 128, sg)
                th = mid.tile([128, NB * 128], BF16, tag="th")
                theta_relu(t, bd1s, 128, th)
                nc.vector.tensor_mul(out=th[:], in0=th[:], in1=sg[:])
                # two bf16-2x halvings (20->10->5 ranks), then reduce 5
                nc.vector.tensor_tensor(out=th[:, 0:1280], in0=th[:, 0:1280],
                                        in1=th[:, 1280:2560], op=ALU.max)
                nc.vector.tensor_tensor(out=th[:, 0:640], in0=th[:, 0:640],
                                        in1=th[:, 640:1280], op=ALU.max)
                mx = sml.tile([128, 128], BF16, tag="mx128")
                nc.vector.tensor_reduce(
                    out=mx[:], in_=bass.AP(th.tensor, th[:].offset,
                                           [[th[:].ap[0][0], 128], [1, 128], [128, 5]]),
                    axis=mybir.AxisListType.X, op=ALU.max)
                acc = sml.tile([128, 32], F32, tag="acc32")
                nc.vector.tensor_reduce(
                    out=acc[:], in_=bass.AP(mx.tensor, mx[:].offset,
                                            [[mx[:].ap[0][0], 128], [1, 32], [32, 4]]),
                    axis=mybir.AxisListType.X, op=ALU.add)
                fp = feat_matmul(t, f1_all[:, t * 17:(t + 1) * 17], 16, wb1s[:], 160)
                f2t = f2_all[:, t * 32:(t + 1) * 32]
                nc.vector.tensor_add(out=acc[:], in0=acc[:], in1=fp[:, 0:32])
                nc.scalar.activation(out=f2t, in_=acc[:], func=AF.Relu)
                # adain stats accumulation
                sp = ps2.tile([1, 64], F32, tag="sp")
                nc.tensor.matmul(out=sp[:, 0:32], lhsT=ones128[:], rhs=f2t, start=True, stop=True)
                f2sq = sml.tile([128, 32], F32, tag="f2sq")
                nc.vector.tensor_mul(out=f2sq[:], in0=f2t, in1=f2t)
                nc.tensor.matmul(out=sp[:, 32:64], lhsT=ones128[:], rhs=f2sq[:], start=True, stop=True)
                nc.vector.tensor_add(out=s1acc[:], in0=s1acc[:], in1=sp[:, 0:32])
                nc.vector.tensor_add(out=s2acc[:], in0=s2acc[:], in1=sp[:, 32:64])
                # style projection h = [tf|1] @ [adain_w; adain_b], cached for
                # pass 1b (independent of the adain stats barrier)
                tft = sml.tile([128, 11], F32, tag="tft")
                nc.sync.dma_start(out=tft[:, 0:10], in_=tf[t * 128:(t + 1) * 128, :])
                nc.vector.memset(tft[:, 10:11], 1.0)
                hp = feat_matmul(t, tft[:], 10, wbas[:], 64)
                nc.scalar.copy(out=hp_all[:, t * 64:(t + 1) * 64], in_=hp[:])

            # ---- adain finalize: mean/rstd broadcast tile ----
            stat = keep.tile([1, 64], F32)
            nc.vector.tensor_scalar_mul(stat[:, 0:32], s1acc[:], 1.0 / V)
            m2 = keep.tile([1, 32], F32)
            nc.vector.tensor_mul(out=m2[:], in0=stat[:, 0:32], in1=s1acc[:])
            nc.vector.tensor_sub(out=m2[:], in0=s2acc[:], in1=m2[:])
            nc.vector.tensor_scalar_mul(m2[:], m2[:], 1.0 / (V - 1))
            nc.scalar.activation(out=m2[:], in_=m2[:], func=AF.Sqrt)
            nc.vector.tensor_scalar_add(m2[:], m2[:], 1e-8)
            nc.vector.reciprocal(out=stat[:, 32:64], in_=m2[:])
            ones1 = keep.tile([1, 128], F32)
            nc.vector.memset(ones1[:], 1.0)
            bc_ps = ps2.tile([128, 64], F32, tag="sp")
            nc.tensor.matmul(out=bc_ps[:], lhsT=ones1[:], rhs=stat[:], start=True, stop=True)
            bc = keep.tile([128, 64], F32)
            nc.scalar.copy(out=bc[:], in_=bc_ps[:])

            # ---- pass 1b: t = adain(f2), dc1 table ----
            for t in range(NT):
                hp = hp_all[:, t * 64:(t + 1) * 64]
                f2t = f2_all[:, t * 32:(t + 1) * 32]
                xn = sml.tile([128, 32], F32, tag="xn")
                nc.vector.tensor_sub(out=xn[:], in0=f2t, in1=bc[:, 0:32])
                nc.vector.tensor_mul(out=xn[:], in0=xn[:], in1=bc[:, 32:64])
                g1 = sml.tile([128, 32], F32, tag="g1")
                nc.scalar.add(out=g1[:], in_=hp[:, 0:32], add=1.0)
                nc.vector.tensor_mul(out=xn[:], in0=xn[:], in1=g1[:])
                tt = t_all[:, t * 33:t * 33 + 32]
                nc.vector.tensor_add(out=tt, in0=xn[:], in1=hp[:, 32:64])
                fp = feat_matmul(t, t_all[:, t * 33:(t + 1) * 33], 32, wbd1s[:], 80, lt_dve=True)
                sup = sml.tile([128, 64], BF16, tag="sup2")
                nc.vector.tensor_copy(out=sup[:], in_=fp[:, 16:80])
                nc.sync.dma_start(
                    out=bass.AP(t_tw[:].tensor, t * 128 * 128, [[128, 128], [1, 64]]),
                    in_=sup[:])

            # ================= pass 2: dc1 -> c1 =================
            for t in range(NT):
                sg128 = sgp.tile([128, NB * 128], BF16, tag="sg")
                gather_ranks(t, t_tw, 128, sg128)
                sg64 = bass.AP(sg128.tensor, sg128[:].offset,
                               [[sg128[:].ap[0][0], 128], [128, NB], [1, 64]])
                th = mid.tile([128, NB * 64], BF16, tag="th")
                theta_relu(t, bd2s, 64, th)
                nc.vector.tensor_tensor(
                    out=th[:].rearrange("p (r c) -> p r c", r=NB, c=64),
                    in0=th[:].rearrange("p (r c) -> p r c", r=NB, c=64),
                    in1=sg64, op=ALU.mult)
                mx = sml.tile([128, 64], BF16, tag="mx128")
                nc.vector.tensor_reduce(
                    out=mx[:], in_=bass.AP(th.tensor, th[:].offset,
                                           [[th[:].ap[0][0], 128], [1, 64], [64, NB]]),
                    axis=mybir.AxisListType.X, op=ALU.max)
                acc = sml.tile([128, 16], F32, tag="acc16")
                nc.vector.tensor_reduce(
                    out=acc[:], in_=bass.AP(mx.tensor, mx[:].offset,
                                            [[mx[:].ap[0][0], 128], [1, 16], [16, 4]]),
                    axis=mybir.AxisListType.X, op=ALU.add)
                fp = feat_matmul(t, t_all[:, t * 33:(t + 1) * 33], 32, wbd1s[:], 80, lt_dve=True)
                c1t = c1_all[:, t * 17:t * 17 + 16]
                nc.vector.tensor_add(out=acc[:], in0=acc[:], in1=fp[:, 0:16])
                nc.scalar.activation(out=c1t, in_=acc[:], func=AF.Relu)
                fp2 = feat_matmul(t, c1_all[:, t * 17:(t + 1) * 17], 16, wbd2s[:], 15, lt_dve=True)
                sup = sml.tile([128, 12], BF16, tag="sup3")
                nc.vector.tensor_copy(out=sup[:], in_=fp2[:, 3:15])
                nc.sync.dma_start(
                    out=bass.AP(t_c1w[:].tensor, t * 128 * 128, [[128, 128], [1, 12]]),
                    in_=sup[:])

            # ================= pass 3: dc2 -> sigmoid -> out =================
            for t in range(NT):
                sg128 = sgp.tile([128, NB * 128], BF16, tag="sg")
                gather_ranks(t, t_c1w, 128, sg128)
                sg12 = bass.AP(sg128.tensor, sg128[:].offset,
                               [[sg128[:].ap[0][0], 128], [128, NB], [1, 12]])
                th = mid.tile([128, NB * 12], BF16, tag="th")
                theta_relu(t, bd3s, 12, th)
                nc.vector.tensor_tensor(
                    out=th[:].rearrange("p (r c) -> p r c", r=NB, c=12),
                    in0=th[:].rearrange("p (r c) -> p r c", r=NB, c=12),
                    in1=sg12, op=ALU.mult)
                mx = sml.tile([128, 12], BF16, tag="mx128")
                nc.vector.tensor_reduce(
                    out=mx[:], in_=bass.AP(th.tensor, th[:].offset,
                                           [[th[:].ap[0][0], 128], [1, 12], [12, NB]]),
                    axis=mybir.AxisListType.X, op=ALU.max)
                acc = sml.tile([128, 3], F32, tag="acc3")
                nc.vector.tensor_reduce(
                    out=acc[:], in_=bass.AP(mx.tensor, mx[:].offset,
                                            [[mx[:].ap[0][0], 128], [1, 3], [3, 4]]),
                    axis=mybir.AxisListType.X, op=ALU.add)
                fp = feat_matmul(t, c1_all[:, t * 17:(t + 1) * 17], 16, wbd2s[:], 15, lt_dve=True)
                nc.vector.tensor_add(out=acc[:], in0=acc[:], in1=fp[:, 0:3])
                sig = sml.tile([128, 3], F32, tag="sig")
                nc.scalar.activation(out=sig[:], in_=acc[:], func=AF.Sigmoid)
                nc.sync.dma_start(out=out[t * 128:(t + 1) * 128, :], in_=sig[:])

    _split_excess_waits(nc)
    lower_extended_insts(nc)
    return nc


_NC_CACHE = None


def kernel(**inputs):
    global _NC_CACHE
    from concourse.bass_utils import run_bass_kernel_spmd

    src = np.ascontiguousarray(np.asarray(inputs['source'], dtype=np.float32))
    tf = np.ascontiguousarray(np.asarray(inputs['target_feature'], dtype=np.float32))
    consts = {
        'bd0': _block_dirs(_normalize_cols(np.asarray(inputs['conv0_dirs'])), 64),
        'bd1': _block_dirs(_normalize_cols(np.asarray(inputs['conv1_dirs'])), 128),
        'bd2': _block_dirs(_normalize_cols(np.asarray(inputs['dc1_dirs'])), 64),
        'bd3': _block_dirs(_normalize_cols(np.asarray(inputs['dc2_dirs'])), 12),
        'wb1': np.vstack([np.asarray(inputs['conv1_w']), np.asarray(inputs['conv1_b'])[None]]).astype(np.float32),
        'wba': np.vstack([np.asarray(inputs['adain_w']), np.asarray(inputs['adain_b'])[None]]).astype(np.float32),
        'wbd1': np.vstack([np.asarray(inputs['dc1_w']), np.asarray(inputs['dc1_b'])[None]]).astype(np.float32),
        'wbd2': np.vstack([np.asarray(inputs['dc2_w']), np.asarray(inputs['dc2_b'])[None]]).astype(np.float32),
    }
    consts = {k: np.ascontiguousarray(v) for k, v in consts.items()}
    if _NC_CACHE is None:
        _NC_CACHE = build_kernel()
    nc = _NC_CACHE
    in_maps = [dict(consts, source=src[b], target_feature=tf[b]) for b in range(B)]
    # sigmoid output can never legitimately be non-finite; a NaN means a
    # transient device-side fault (observed ~once per ~30 runs on this shared
    # axon terminal) — retry the execution, the compiled NEFF is cached
    for attempt in range(3):
        res = run_bass_kernel_spmd(nc, in_maps, list(range(B)))
        outp = np.stack([res.results[b]['out'] for b in range(B)]).astype(np.float32)
        if np.isfinite(outp).all():
            break
    return outp


if __name__ == '__main__':
    inp = dict(np.load('/root/problem/dev/inputs.npz'))
    o = kernel(**inp)
    print(o.shape, o.dtype)

